# revision 1
# baseline (speedup 1.0000x reference)
"""Bass/Trainium2 kernel for nn_ConcatenationFusionLayer_29575144801128.

Math: out = inputs.reshape(65536, 1024) where inputs is a contiguous
(65536, 2, 512) f32 tensor -- i.e. the output bytes are identical to the
input bytes.  The kernel is therefore a pure HBM->HBM memcpy, done
data-parallel across 8 NeuronCores (batch dim sharded, 8192 rows = 32 MiB
per core).  Each core issues chunked DRAM->DRAM DMA copies (no SBUF
round-trip needed), split across the two HWDGE rings (sync + scalar).
"""

import numpy as np

N_CORES = 8
B = 65536
FLAT = 1024  # 2 * 512
PER_CORE = B // N_CORES  # 8192 rows -> 32 MiB per core

# Number of dma_start chunks per core; even chunks go on nc.sync's HWDGE
# ring, odd chunks on nc.scalar's.  Two rings keep all 16 SDMA engines fed
# across per-DMA completion stalls (engines round-robin between rings at
# packet granularity); 16 chunks x 2 MiB measured best cold-start.
N_CHUNKS = 16

_cache = {}


def _build_nc():
    import concourse.bass as bass
    import concourse.mybir as mybir

    nc = bass.Bass()
    x = nc.declare_dram_parameter(
        "x", [PER_CORE, FLAT], mybir.dt.float32, isOutput=False
    )
    y = nc.declare_dram_parameter(
        "y", [PER_CORE, FLAT], mybir.dt.float32, isOutput=True
    )

    total = PER_CORE * FLAT  # elements per core
    assert total % N_CHUNKS == 0
    chunk = total // N_CHUNKS

    with (
        nc.Block() as block,
        nc.semaphore("dma_sem") as dma_sem,
    ):

        @block.sync
        def _(sync):
            for i in range(0, N_CHUNKS, 2):
                sync.dma_start(
                    out=bass.AP(y, i * chunk, [[1, chunk]]),
                    in_=bass.AP(x, i * chunk, [[1, chunk]]),
                ).then_inc(dma_sem, 16)
            # wait for ALL chunks (both engines' DMAs) to land
            sync.wait_ge(dma_sem, 16 * N_CHUNKS)

        @block.scalar
        def _(scalar):
            for i in range(1, N_CHUNKS, 2):
                scalar.dma_start(
                    out=bass.AP(y, i * chunk, [[1, chunk]]),
                    in_=bass.AP(x, i * chunk, [[1, chunk]]),
                ).then_inc(dma_sem, 16)

    return nc


def _run(inputs_arr: np.ndarray, **spmd_kwargs):
    """Shard, run on 8 cores, gather.  Returns (out, BassKernelResults)."""
    from concourse.bass_utils import run_bass_kernel_spmd

    x = np.ascontiguousarray(np.asarray(inputs_arr, dtype=np.float32))
    assert x.shape == (B, 2, 512), x.shape
    shards = x.reshape(N_CORES, PER_CORE, FLAT)

    if "nc" not in _cache:
        _cache["nc"] = _build_nc()
    nc = _cache["nc"]

    in_maps = [{"x": shards[i]} for i in range(N_CORES)]
    res = run_bass_kernel_spmd(nc, in_maps, core_ids=list(range(N_CORES)), **spmd_kwargs)
    out = np.concatenate([r["y"] for r in res.results], axis=0)
    return out, res


def kernel(**inputs) -> np.ndarray:
    out, _ = _run(inputs["inputs"])
    return out



# revision 2
# speedup vs baseline: 1.1849x; 1.1849x over previous
"""Bass/Trainium2 kernel for nn_ConcatenationFusionLayer_29575144801128.

Math: out = inputs.reshape(65536, 1024) where inputs is a contiguous
(65536, 2, 512) f32 tensor -- i.e. the output bytes are identical to the
input bytes.  The kernel is therefore a pure HBM->HBM memcpy, done
data-parallel across 8 NeuronCores (batch dim sharded, 8192 rows = 32 MiB
per core).  Each core issues chunked DRAM->DRAM DMA copies (no SBUF
round-trip needed), split across the two HWDGE rings (sync + scalar).

Measured roofline (ntff traces, 2026-08): each InstDMACopy is split by the
HWDGE into 16 equal contiguous per-engine portions regardless of AP
descriptor structure, and each SDMA engine sustains ~20.8 GB/s of
DRAM->DRAM copy (min packet 2.42 us/64 KiB = port rate; median 3.15 us --
HBM-latency bubbles inside packet processing).  A core running SOLO on the
chip shows the same 20.8 GB/s/engine, so the cap is engine-internal, not
HBM contention.  Copy phase floor = 32 MiB / (16 x 20.8 GB/s) = 100.6 us;
program prologue ~7 us + first-byte ~1.3 us + completion/barrier tail
~3.5 us puts exec at ~112-114 us, which this kernel hits.  Variants that
tie within noise: 2x16MiB single DMA per ring, 3rd SWDGE ring, gpsimd
drain skip, no partition id, 12/15/17-descriptor chunking, 32K-64K packet
sizes.  Known intermittent mode (environmental, epoch-clustered, ~30-60%%
of runs): SDMA engine 15 degrades to ~17.5 GB/s (+20 us tail); per-DMA
even split makes static rebalancing impossible.  Dead ends measured:
SBUF bounce (2x engine transits), collectives (verifier rejects I/O
tensors; bounce re-adds traffic), >64 KiB descriptors (16-bit ISA field),
strided half-precision reads/writes (HBM granularity + RMW).
"""

import numpy as np

N_CORES = 8
B = 65536
FLAT = 1024  # 2 * 512
PER_CORE = B // N_CORES  # 8192 rows -> 32 MiB per core

# Number of dma_start chunks per core; even chunks go on nc.sync's HWDGE
# ring, odd chunks on nc.scalar's.  Two rings keep all 16 SDMA engines fed
# across per-DMA completion stalls (engines round-robin between rings at
# packet granularity); 16 chunks x 2 MiB measured best cold-start.
N_CHUNKS = 16

_cache = {}


def _build_nc():
    import concourse.bass as bass
    import concourse.mybir as mybir

    nc = bass.Bass()
    x = nc.declare_dram_parameter(
        "x", [PER_CORE, FLAT], mybir.dt.float32, isOutput=False
    )
    y = nc.declare_dram_parameter(
        "y", [PER_CORE, FLAT], mybir.dt.float32, isOutput=True
    )

    total = PER_CORE * FLAT  # elements per core
    assert total % N_CHUNKS == 0
    chunk = total // N_CHUNKS

    with (
        nc.Block() as block,
        nc.semaphore("dma_sem") as dma_sem,
    ):

        @block.sync
        def _(sync):
            for i in range(0, N_CHUNKS, 2):
                sync.dma_start(
                    out=bass.AP(y, i * chunk, [[1, chunk]]),
                    in_=bass.AP(x, i * chunk, [[1, chunk]]),
                ).then_inc(dma_sem, 16)
            # wait for ALL chunks (both engines' DMAs) to land
            sync.wait_ge(dma_sem, 16 * N_CHUNKS)

        @block.scalar
        def _(scalar):
            for i in range(1, N_CHUNKS, 2):
                scalar.dma_start(
                    out=bass.AP(y, i * chunk, [[1, chunk]]),
                    in_=bass.AP(x, i * chunk, [[1, chunk]]),
                ).then_inc(dma_sem, 16)

    return nc


def _run(inputs_arr: np.ndarray, **spmd_kwargs):
    """Shard, run on 8 cores, gather.  Returns (out, BassKernelResults)."""
    from concourse.bass_utils import run_bass_kernel_spmd

    x = np.ascontiguousarray(np.asarray(inputs_arr, dtype=np.float32))
    assert x.shape == (B, 2, 512), x.shape
    shards = x.reshape(N_CORES, PER_CORE, FLAT)

    if "nc" not in _cache:
        _cache["nc"] = _build_nc()
    nc = _cache["nc"]

    in_maps = [{"x": shards[i]} for i in range(N_CORES)]
    res = run_bass_kernel_spmd(nc, in_maps, core_ids=list(range(N_CORES)), **spmd_kwargs)
    out = np.concatenate([r["y"] for r in res.results], axis=0)
    return out, res


def kernel(**inputs) -> np.ndarray:
    out, _ = _run(inputs["inputs"])
    return out



# revision 3
# speedup vs baseline: 1.1880x; 1.0026x over previous
"""Bass/Trainium2 kernel for nn_ConcatenationFusionLayer_29575144801128.

Math: out = inputs.reshape(65536, 1024) where inputs is a contiguous
(65536, 2, 512) f32 tensor -- i.e. the output bytes are identical to the
input bytes.  The kernel is therefore a pure HBM->HBM memcpy, done
data-parallel across 8 NeuronCores (batch dim sharded, 8192 rows = 32 MiB
per core).  Each core issues chunked DRAM->DRAM DMA copies (no SBUF
round-trip needed), split across the two HWDGE rings (sync + scalar).

Measured roofline (ntff traces, 2026-08): each InstDMACopy is split by the
HWDGE into 16 equal contiguous per-engine portions regardless of AP
descriptor structure, and each SDMA engine sustains ~20.8 GB/s of
DRAM->DRAM copy (min packet 2.42 us/64 KiB = port rate; median 3.15 us --
HBM-latency bubbles inside packet processing).  A core running SOLO on the
chip shows the same 20.8 GB/s/engine, so the cap is engine-internal, not
HBM contention.  Copy phase floor = 32 MiB / (16 x 20.8 GB/s) = 100.6 us;
program prologue ~7 us + first-byte ~1.3 us + completion/barrier tail
~3.5 us puts exec at ~112-114 us, which this kernel hits.  Variants that
tie within noise: 2x16MiB single DMA per ring, 3rd SWDGE ring, gpsimd
drain skip, no partition id, 12/15/17-descriptor chunking, 32K-64K packet
sizes.  Known intermittent mode (environmental, epoch-clustered, ~30-60%
of runs): SDMA engine 15 degrades to ~17.5 GB/s (+20 us tail); per-DMA
even split makes static rebalancing impossible.  Dead ends measured:
SBUF bounce (2x engine transits), collectives (verifier rejects I/O
tensors; bounce re-adds traffic), >64 KiB descriptors (16-bit ISA field),
strided half-precision reads/writes (HBM granularity + RMW).
"""

import numpy as np

N_CORES = 8
B = 65536
FLAT = 1024  # 2 * 512
PER_CORE = B // N_CORES  # 8192 rows -> 32 MiB per core

# Number of dma_start chunks per core; even chunks go on nc.sync's HWDGE
# ring, odd chunks on nc.scalar's.  Two rings keep all 16 SDMA engines fed
# across per-DMA completion stalls (engines round-robin between rings at
# packet granularity); 16 chunks x 2 MiB measured best cold-start.
N_CHUNKS = 16

_cache = {}


def _build_nc():
    import concourse.bass as bass
    import concourse.mybir as mybir

    nc = bass.Bass()
    x = nc.declare_dram_parameter(
        "x", [PER_CORE, FLAT], mybir.dt.float32, isOutput=False
    )
    y = nc.declare_dram_parameter(
        "y", [PER_CORE, FLAT], mybir.dt.float32, isOutput=True
    )

    total = PER_CORE * FLAT  # elements per core
    assert total % N_CHUNKS == 0
    chunk = total // N_CHUNKS

    with (
        nc.Block() as block,
        nc.semaphore("dma_sem") as dma_sem,
    ):

        @block.sync
        def _(sync):
            for i in range(0, N_CHUNKS, 2):
                sync.dma_start(
                    out=bass.AP(y, i * chunk, [[1, chunk]]),
                    in_=bass.AP(x, i * chunk, [[1, chunk]]),
                ).then_inc(dma_sem, 16)
            # wait for ALL chunks (both engines' DMAs) to land
            sync.wait_ge(dma_sem, 16 * N_CHUNKS)

        @block.scalar
        def _(scalar):
            for i in range(1, N_CHUNKS, 2):
                scalar.dma_start(
                    out=bass.AP(y, i * chunk, [[1, chunk]]),
                    in_=bass.AP(x, i * chunk, [[1, chunk]]),
                ).then_inc(dma_sem, 16)

    return nc


def _run(inputs_arr: np.ndarray, **spmd_kwargs):
    """Shard, run on 8 cores, gather.  Returns (out, BassKernelResults)."""
    from concourse.bass_utils import run_bass_kernel_spmd

    x = np.ascontiguousarray(np.asarray(inputs_arr, dtype=np.float32))
    assert x.shape == (B, 2, 512), x.shape
    shards = x.reshape(N_CORES, PER_CORE, FLAT)

    if "nc" not in _cache:
        _cache["nc"] = _build_nc()
    nc = _cache["nc"]

    in_maps = [{"x": shards[i]} for i in range(N_CORES)]
    res = run_bass_kernel_spmd(nc, in_maps, core_ids=list(range(N_CORES)), **spmd_kwargs)
    out = np.concatenate([r["y"] for r in res.results], axis=0)
    return out, res


def kernel(**inputs) -> np.ndarray:
    out, _ = _run(inputs["inputs"])
    return out

